# revision 5
# baseline (speedup 1.0000x reference)
"""Trainium2 Bass kernel for nn_BioClassifier: whitening + sequential Oja scan + readout.

Algorithm: chunk the 2048-sample sequential Oja scan into 16 blocks of K=128.
Within a block starting at weight W0 with whitened samples X [D,K] (XT [K,D]):
    Y = Y0 A,  U = (X - T0 A) B
    A = (I - lr*SU(C))^-1,  B = (I + lr*SU(G))^-1
    C = U^T X,  G = Y^T Y,   (SU = strict upper triangle)
and the key identity T0^T X = Y0^T W0 X = Y0^T Y0 = Syy, so the (A,B,C,G)
fixed point closes over K x K matrices only:
    S  = Sxx - A^T Syy
    CT = S^T B              (computed transposed: lhsT = S)
    A  = I + (lr*SL(CT))^T A
    G  = A^T (Syy A)
    B  = I - (lr*SL(G))^T B (G symmetric)
Iterated RING_ITERS times (geometric convergence ~0.37/iter), then:
    YT = A^T Y0T (feats), UT = B^T (XT - A^T T0T)
    W += lr * YT^T UT, WT += lr * UT^T YT  (both layouts kept in SBUF)
    logits = relu(Y0 A)^T-path @ readout + bias
Whitening uses xw = xc + xc @ (M - I)^T with (M - I) in bf16 (M ~ I + 0.01*N,
so the bf16 term is a small correction; xc added back in fp32).
All heavy compute in fp32 matmuls unless MM_DT overrides.
"""

import os
import sys
from contextlib import ExitStack

sys.path.insert(0, "/opt/trn_rl_repo")

import numpy as np
import ml_dtypes

import concourse.bass as bass
import concourse.mybir as mybir
from concourse.tile import TileContext
from concourse.masks import make_identity
from concourse.bass_utils import run_bass_kernel_spmd
from concourse.vector_clock import ScopedClock

LR = 1e-3
B, D, H, O = 2048, 784, 256, 10
K = 128
NBLK = B // K
DP, DC = 112, 7          # D = 784 = 7 * 112
HP, HC = 128, 2          # H = 256 = 2 * 128
DS = D // 2              # 392: matmul free-dim split for D-wide outputs

RING_ITERS = int(os.environ.get("RING_ITERS", "7"))
N_CORES = 8

f32 = mybir.dt.float32
bf16 = mybir.dt.bfloat16

def _install_ntff_hook():
    """The agent image's `antenv` lacks `axon_hooks`, so trace=True degrades.
    Synthesize the module and register the ctypes NTFF hook from trn_boot."""
    import types
    import antenv

    if getattr(antenv, "axon_hooks", None) is not None:
        return
    mod = types.ModuleType("antenv.axon_hooks")
    _hook_box = [None]
    mod.set_axon_ntff_profile_hook = lambda h: _hook_box.__setitem__(0, h)
    mod.get_axon_ntff_profile_hook = lambda: _hook_box[0]
    sys.modules["antenv.axon_hooks"] = mod
    antenv.axon_hooks = mod
    try:
        sys.path.insert(0, "/root/.axon_site")
        from trn_agent_boot.trn_boot import _ntff_profile_via_ctypes

        hook = _ntff_profile_via_ctypes("/opt/axon/libaxon_pjrt.so")
        if hook is not None:
            mod.set_axon_ntff_profile_hook(hook)
    except Exception:
        pass


try:
    _install_ntff_hook()
except Exception:
    pass

_drain_patched = False


def _patch_drain():
    """This walrus build only supports one sync-wait per CTRL instruction;
    split the Tile kernel-tail drain into one drain per semaphore wait."""
    global _drain_patched
    if _drain_patched:
        return

    def patched(self, tick_clock, wait_clock):
        drain_inst = self.nc.sync.drain()
        wait_clock.add_sem_waits(
            drain_inst.ins, ScopedClock({None: tick_clock.global_clock})
        )
        mi = drain_inst.ins
        si = mi.sync_info
        if si is not None and len(si.on_wait) > 1:
            waits = list(si.on_wait)
            mi.sync_info = mybir.SyncInfo(
                on_wait=[waits[0]], on_update=list(si.on_update)
            )
            for w in waits[1:]:
                d2 = self.nc.sync.drain()
                d2.ins.sync_info = mybir.SyncInfo(on_wait=[w], on_update=[])
        self.nc.all_engine_barrier()
        assert self.sems is not None
        popped = self.nc._tile_sem_poison_stack.pop()
        assert popped is self._sem_poison
        self.nc.clear_and_free_semaphores(list(self.sems.allocated().values()))
        self.nc.all_engine_barrier()

    TileContext._drain_and_barrier = patched
    _drain_patched = True


def _split_multiwait(nc, limit=1):
    """This walrus build supports only `limit` sync-waits per instruction.
    Hoist extra waits onto NoOps inserted just before, in the same engine
    stream (engines are in-order, so earlier waits are strictly safe)."""
    n_split = 0
    for f in nc.m.functions:
        for bb in f.blocks:
            insts = list(bb.instructions)
            if not any(
                i.sync_info is not None and len(i.sync_info.on_wait) > limit
                for i in insts
            ):
                continue
            new = []
            for inst in insts:
                si = inst.sync_info
                if si is not None and len(si.on_wait) > limit:
                    waits = list(si.on_wait)
                    for j, w in enumerate(waits[: len(waits) - limit]):
                        nop = mybir.InstNoOp(
                            name=f"{inst.name}-hw{j}", engine=inst.engine,
                            ins=[], outs=[],
                        )
                        nop.sync_info = mybir.SyncInfo(on_wait=[w], on_update=[])
                        new.append(nop)
                        n_split += 1
                    inst.sync_info = mybir.SyncInfo(
                        on_wait=waits[len(waits) - limit:],
                        on_update=list(si.on_update),
                    )
                new.append(inst)
            bb.instructions = new
    return n_split


def build_nc(ring_iters=RING_ITERS):
    _patch_drain()
    nc = bass.Bass()
    x_d = nc.dram_tensor("x", [B, D], f32, kind="ExternalInput")
    mu_d = nc.dram_tensor("mu_b", [128, D], f32, kind="ExternalInput")
    pt_d = nc.dram_tensor("pt", [DP, DC, D], bf16, kind="ExternalInput")
    w_d = nc.dram_tensor("w", [HP, HC, D], f32, kind="ExternalInput")
    wt_d = nc.dram_tensor("wt", [DP, DC, H], f32, kind="ExternalInput")
    rt_d = nc.dram_tensor("rt", [HP, HC, O], f32, kind="ExternalInput")
    bb_d = nc.dram_tensor("b_b", [128, O], f32, kind="ExternalInput")
    out_d = nc.dram_tensor("out", [B, O], f32, kind="ExternalOutput")

    AT = mybir.AluOpType

    with TileContext(nc) as tc, ExitStack() as ctx:
        persist = ctx.enter_context(tc.tile_pool(name="persist", bufs=1))
        xpool = ctx.enter_context(tc.tile_pool(name="xpool", bufs=3))
        small = ctx.enter_context(tc.tile_pool(name="small", bufs=2))
        psA = ctx.enter_context(tc.tile_pool(name="psA", bufs=3, space="PSUM"))
        psB = ctx.enter_context(tc.tile_pool(name="psB", bufs=3, space="PSUM"))
        psT = ctx.enter_context(tc.tile_pool(name="psT", bufs=2, space="PSUM"))

        ident = persist.tile([128, 128], f32, tag="ident")
        make_identity(nc, ident)
        maskSL = persist.tile([K, K], f32, tag="maskSL")
        nc.gpsimd.memset(maskSL, LR)
        # keep lr where row > col (strict lower), else 0
        nc.gpsimd.affine_select(
            out=maskSL, in_=maskSL, compare_op=AT.is_gt, fill=0.0,
            base=0, pattern=[[-1, K]], channel_multiplier=1,
        )

        mu_t = persist.tile([128, D], f32, tag="mu")
        nc.sync.dma_start(out=mu_t, in_=mu_d[:, :])
        pt_t = persist.tile([DP, DC, D], bf16, tag="pt")
        nc.sync.dma_start(out=pt_t, in_=pt_d[:, :, :])
        W = persist.tile([HP, HC, D], f32, tag="W")
        nc.sync.dma_start(out=W, in_=w_d[:, :, :])
        WT = persist.tile([DP, DC, H], f32, tag="WT")
        nc.sync.dma_start(out=WT, in_=wt_d[:, :, :])
        RT = persist.tile([HP, HC, O], f32, tag="RT")
        nc.sync.dma_start(out=RT, in_=rt_d[:, :, :])
        bb = persist.tile([128, O], f32, tag="bb")
        nc.sync.dma_start(out=bb, in_=bb_d[:, :])

        Xall = persist.tile([DP, NBLK, DC, K], f32, tag="Xall")
        XTall = persist.tile([K, NBLK, D], f32, tag="XTall")
        Sxxall = persist.tile([K, NBLK, K], f32, tag="Sxxall")

        # ---------------- whitening ----------------
        for bi in range(NBLK):
            xt = xpool.tile([128, D], f32, tag="xraw")
            nc.sync.dma_start(out=xt, in_=x_d[bi * K:(bi + 1) * K, :])
            xc = xpool.tile([128, D], f32, tag="xc")
            nc.vector.tensor_sub(xc, xt, mu_t)
            xctb = xpool.tile([DP, DC, K], bf16, tag="xct")
            for ic in range(DC):
                tp = psT.tile([DP, K], f32, tag="t")
                nc.tensor.transpose(tp, xc[:, ic * DP:(ic + 1) * DP], ident)
                nc.vector.tensor_copy(xctb[:, ic, :], tp)
            XTb = XTall[:, bi, :]
            for s in range(2):
                ps = psB.tile([K, DS], f32, tag="big")
                for ic in range(DC):
                    nc.tensor.matmul(
                        ps, xctb[:, ic, :], pt_t[:, ic, s * DS:(s + 1) * DS],
                        start=(ic == 0), stop=(ic == DC - 1),
                    )
                nc.vector.tensor_add(
                    XTb[:, s * DS:(s + 1) * DS], ps, xc[:, s * DS:(s + 1) * DS]
                )
            for ic in range(DC):
                tp = psT.tile([DP, K], f32, tag="t")
                nc.tensor.transpose(tp, XTb[:, ic * DP:(ic + 1) * DP], ident)
                nc.vector.tensor_copy(Xall[:, bi, ic, :], tp)
            ps = psA.tile([K, K], f32, tag="kk")
            for ic in range(DC):
                nc.tensor.matmul(
                    ps, Xall[:, bi, ic, :], Xall[:, bi, ic, :],
                    start=(ic == 0), stop=(ic == DC - 1),
                )
            nc.vector.tensor_copy(Sxxall[:, bi, :], ps)

        # ---------------- sequential block scan ----------------
        for bi in range(NBLK):
            XTb = XTall[:, bi, :]
            # Y0 [HP, HC, K] = W X
            y0 = small.tile([HP, HC, K], f32, tag="y0")
            for hc in range(HC):
                ps = psA.tile([HP, K], f32, tag="kk")
                for ic in range(DC):
                    nc.tensor.matmul(
                        ps, WT[:, ic, hc * HP:(hc + 1) * HP], Xall[:, bi, ic, :],
                        start=(ic == 0), stop=(ic == DC - 1),
                    )
                nc.vector.tensor_copy(y0[:, hc, :], ps)
            # Y0T [K, H] = X^T W^T
            y0t = small.tile([K, H], f32, tag="y0t")
            ps = psB.tile([K, H], f32, tag="big")
            for ic in range(DC):
                nc.tensor.matmul(
                    ps, Xall[:, bi, ic, :], WT[:, ic, :],
                    start=(ic == 0), stop=(ic == DC - 1),
                )
            nc.vector.tensor_copy(y0t, ps)
            # Syy [K, K] = Y0^T Y0
            syy = small.tile([K, K], f32, tag="syy")
            ps = psA.tile([K, K], f32, tag="kk")
            for hc in range(HC):
                nc.tensor.matmul(
                    ps, y0[:, hc, :], y0[:, hc, :],
                    start=(hc == 0), stop=(hc == HC - 1),
                )
            nc.vector.tensor_copy(syy, ps)
            # T0T [K, D] = Y0^T W
            t0t = small.tile([K, D], f32, tag="t0t")
            for s in range(2):
                ps = psB.tile([K, DS], f32, tag="big")
                for hc in range(HC):
                    nc.tensor.matmul(
                        ps, y0[:, hc, :], W[:, hc, s * DS:(s + 1) * DS],
                        start=(hc == 0), stop=(hc == HC - 1),
                    )
                nc.vector.tensor_copy(t0t[:, s * DS:(s + 1) * DS], ps)

            sxx = Sxxall[:, bi, :]
            # ---- fixed-point ring on K x K matrices ----
            A = small.tile([K, K], f32, tag="A")
            Bm = small.tile([K, K], f32, tag="B")
            nc.vector.tensor_copy(A, ident)
            nc.vector.tensor_copy(Bm, ident)
            for m in range(ring_iters):
                r1 = psA.tile([K, K], f32, tag="kk")
                nc.tensor.matmul(r1, A, syy, start=True, stop=True)
                s_sb = small.tile([K, K], f32, tag="S")
                nc.vector.tensor_sub(s_sb, sxx, r1)
                ct = psA.tile([K, K], f32, tag="kk")
                nc.tensor.matmul(ct, s_sb, Bm, start=True, stop=True)
                nt = small.tile([K, K], f32, tag="NT")
                nc.vector.tensor_mul(nt, ct, maskSL)
                a1 = psA.tile([K, K], f32, tag="kk")
                nc.tensor.matmul(a1, nt, A, start=True, stop=True)
                nc.vector.tensor_add(A, a1, ident)
                z2 = psA.tile([K, K], f32, tag="kk")
                nc.tensor.matmul(z2, syy, A, start=True, stop=True)
                z2s = small.tile([K, K], f32, tag="Z2")
                nc.vector.tensor_copy(z2s, z2)
                g = psA.tile([K, K], f32, tag="kk")
                nc.tensor.matmul(g, A, z2s, start=True, stop=True)
                gm = small.tile([K, K], f32, tag="GM")
                nc.vector.tensor_mul(gm, g, maskSL)
                b1 = psA.tile([K, K], f32, tag="kk")
                nc.tensor.matmul(b1, gm, Bm, start=True, stop=True)
                nc.vector.tensor_sub(Bm, ident, b1)

            # ---- epilogue ----
            yt = small.tile([K, H], f32, tag="yt")
            ps = psB.tile([K, H], f32, tag="big")
            nc.tensor.matmul(ps, A, y0t, start=True, stop=True)
            nc.vector.tensor_copy(yt, ps)
            # logits: relu(Y0 A) then readout
            relu_y = small.tile([HP, HC, K], f32, tag="reluy")
            for hc in range(HC):
                ps2 = psA.tile([HP, K], f32, tag="kk")
                nc.tensor.matmul(
                    ps2, y0t[:, hc * HP:(hc + 1) * HP], A, start=True, stop=True
                )
                nc.scalar.activation(
                    relu_y[:, hc, :], ps2, mybir.ActivationFunctionType.Relu
                )
            lg = psA.tile([K, O], f32, tag="kk")
            for hc in range(HC):
                nc.tensor.matmul(
                    lg, relu_y[:, hc, :], RT[:, hc, :],
                    start=(hc == 0), stop=(hc == HC - 1),
                )
            lgs = small.tile([K, O], f32, tag="lg")
            nc.vector.tensor_add(lgs, lg, bb)
            nc.sync.dma_start(out=out_d[bi * K:(bi + 1) * K, :], in_=lgs)

            # UT = B^T (XT - A^T T0T)
            q = small.tile([K, D], f32, tag="q")
            for s in range(2):
                ps3 = psB.tile([K, DS], f32, tag="big")
                nc.tensor.matmul(
                    ps3, A, t0t[:, s * DS:(s + 1) * DS], start=True, stop=True
                )
                nc.vector.tensor_sub(
                    q[:, s * DS:(s + 1) * DS], XTb[:, s * DS:(s + 1) * DS], ps3
                )
            ut = small.tile([K, D], f32, tag="ut")
            for s in range(2):
                ps4 = psB.tile([K, DS], f32, tag="big")
                nc.tensor.matmul(
                    ps4, Bm, q[:, s * DS:(s + 1) * DS], start=True, stop=True
                )
                nc.vector.tensor_copy(ut[:, s * DS:(s + 1) * DS], ps4)
            # W += lr * Y U^T ; WT += lr * U Y^T
            for hc in range(HC):
                for s in range(2):
                    ps5 = psB.tile([HP, DS], f32, tag="big")
                    nc.tensor.matmul(
                        ps5, yt[:, hc * HP:(hc + 1) * HP],
                        ut[:, s * DS:(s + 1) * DS], start=True, stop=True,
                    )
                    wslice = W[:, hc, s * DS:(s + 1) * DS]
                    nc.vector.scalar_tensor_tensor(
                        wslice, ps5, LR, wslice, op0=AT.mult, op1=AT.add
                    )
            for dc in range(DC):
                ps6 = psB.tile([DP, H], f32, tag="big")
                nc.tensor.matmul(
                    ps6, ut[:, dc * DP:(dc + 1) * DP], yt, start=True, stop=True
                )
                wtslice = WT[:, dc, :]
                nc.vector.scalar_tensor_tensor(
                    wtslice, ps6, LR, wtslice, op0=AT.mult, op1=AT.add
                )

    _split_multiwait(nc)
    return nc


def prep_inputs(x, whiten_mean, whiten_mat, oja_W, readout_W, readout_b):
    """Host-side layout/dtype prep (no contractions)."""
    x = np.ascontiguousarray(x, dtype=np.float32)
    mu_b = np.broadcast_to(
        np.asarray(whiten_mean, dtype=np.float32)[None, :], (128, D)
    ).copy()
    P = np.asarray(whiten_mat, dtype=np.float32) - np.eye(D, dtype=np.float32)
    # pt[dp, ic, dout] = P^T[ic*112+dp, dout] = P[dout, ic*112+dp]
    pt = np.ascontiguousarray(
        P.T.reshape(DC, DP, D).transpose(1, 0, 2).astype(ml_dtypes.bfloat16)
    )
    Wf = np.asarray(oja_W, dtype=np.float32)
    w = np.ascontiguousarray(Wf.reshape(HC, HP, D).transpose(1, 0, 2))
    wt = np.ascontiguousarray(Wf.T.reshape(DC, DP, H).transpose(1, 0, 2))
    Rf = np.asarray(readout_W, dtype=np.float32)
    rt = np.ascontiguousarray(Rf.T.reshape(HC, HP, O).transpose(1, 0, 2))
    b_b = np.broadcast_to(
        np.asarray(readout_b, dtype=np.float32)[None, :], (128, O)
    ).copy()
    return {
        "x": x, "mu_b": mu_b, "pt": pt, "w": w, "wt": wt, "rt": rt, "b_b": b_b
    }


_cached_nc = None


def _get_nc():
    global _cached_nc
    if _cached_nc is None:
        _cached_nc = build_nc()
    return _cached_nc


def kernel(x, whiten_mean, whiten_mat, oja_W, readout_W, readout_b, **run_kwargs):
    nc = _get_nc()
    ins = prep_inputs(x, whiten_mean, whiten_mat, oja_W, readout_W, readout_b)
    res = run_bass_kernel_spmd(
        nc, [ins] * N_CORES, core_ids=list(range(N_CORES)), **run_kwargs
    )
    out = res.results[0]["out"]
    if run_kwargs:
        kernel.last_result = res
    return out


# revision 6
# speedup vs baseline: 1.7103x; 1.7103x over previous
"""Trainium2 Bass kernel for nn_BioClassifier: whitening + sequential Oja scan + readout.

Algorithm: chunk the 2048-sample sequential Oja scan into 16 blocks of K=128.
Within a block starting at weight W0 with whitened samples X [D,K] (XT [K,D]):
    Y = Y0 A,  U = (X - T0 A) B
    A = (I - lr*SU(C))^-1,  B = (I + lr*SU(G))^-1
    C = U^T X,  G = Y^T Y,   (SU = strict upper triangle)
and the key identity T0^T X = Y0^T W0 X = Y0^T Y0 = Syy, so the (A,B,C,G)
fixed point closes over K x K matrices only:
    S  = Sxx - A^T Syy
    CT = S^T B              (computed transposed: lhsT = S)
    A  = I + (lr*SL(CT))^T A
    G  = A^T (Syy A)
    B  = I - (lr*SL(G))^T B (G symmetric)
Iterated RING_ITERS times (geometric convergence ~0.37/iter), then:
    YT = A^T Y0T (feats), UT = B^T (XT - A^T T0T)
    W += lr * YT^T UT, WT += lr * UT^T YT  (both layouts kept in SBUF)
    logits = relu(Y0 A)^T-path @ readout + bias
Whitening uses xw = xc + xc @ (M - I)^T with (M - I) in bf16 (M ~ I + 0.01*N,
so the bf16 term is a small correction; xc added back in fp32).
All heavy compute in fp32 matmuls unless MM_DT overrides.
"""

import os
import sys
from contextlib import ExitStack

sys.path.insert(0, "/opt/trn_rl_repo")

import numpy as np
import ml_dtypes

import concourse.bass as bass
import concourse.mybir as mybir
from concourse.tile import TileContext
from concourse.masks import make_identity
from concourse.bass_utils import run_bass_kernel_spmd
from concourse.vector_clock import ScopedClock

LR = 1e-3
B, D, H, O = 2048, 784, 256, 10
K = 128
NBLK = B // K
DP, DC = 112, 7          # D = 784 = 7 * 112
HP, HC = 128, 2          # H = 256 = 2 * 128
DS = D // 2              # 392: matmul free-dim split for D-wide outputs

RING_ITERS = int(os.environ.get("RING_ITERS", "5"))
N_CORES = 8

f32 = mybir.dt.float32
bf16 = mybir.dt.bfloat16

def _install_ntff_hook():
    """The agent image's `antenv` lacks `axon_hooks`, so trace=True degrades.
    Synthesize the module and register the ctypes NTFF hook from trn_boot."""
    import types
    import antenv

    if getattr(antenv, "axon_hooks", None) is not None:
        return
    mod = types.ModuleType("antenv.axon_hooks")
    _hook_box = [None]
    mod.set_axon_ntff_profile_hook = lambda h: _hook_box.__setitem__(0, h)
    mod.get_axon_ntff_profile_hook = lambda: _hook_box[0]
    sys.modules["antenv.axon_hooks"] = mod
    antenv.axon_hooks = mod
    try:
        sys.path.insert(0, "/root/.axon_site")
        from trn_agent_boot.trn_boot import _ntff_profile_via_ctypes

        hook = _ntff_profile_via_ctypes("/opt/axon/libaxon_pjrt.so")
        if hook is not None:
            mod.set_axon_ntff_profile_hook(hook)
    except Exception:
        pass


try:
    _install_ntff_hook()
except Exception:
    pass

_drain_patched = False


def _patch_drain():
    """This walrus build only supports one sync-wait per CTRL instruction;
    split the Tile kernel-tail drain into one drain per semaphore wait."""
    global _drain_patched
    if _drain_patched:
        return

    def patched(self, tick_clock, wait_clock):
        drain_inst = self.nc.sync.drain()
        wait_clock.add_sem_waits(
            drain_inst.ins, ScopedClock({None: tick_clock.global_clock})
        )
        mi = drain_inst.ins
        si = mi.sync_info
        if si is not None and len(si.on_wait) > 1:
            waits = list(si.on_wait)
            mi.sync_info = mybir.SyncInfo(
                on_wait=[waits[0]], on_update=list(si.on_update)
            )
            for w in waits[1:]:
                d2 = self.nc.sync.drain()
                d2.ins.sync_info = mybir.SyncInfo(on_wait=[w], on_update=[])
        self.nc.all_engine_barrier()
        assert self.sems is not None
        popped = self.nc._tile_sem_poison_stack.pop()
        assert popped is self._sem_poison
        self.nc.clear_and_free_semaphores(list(self.sems.allocated().values()))
        self.nc.all_engine_barrier()

    TileContext._drain_and_barrier = patched
    _drain_patched = True


def _split_multiwait(nc, limit=1):
    """This walrus build supports only `limit` sync-waits per instruction.
    Hoist extra waits onto NoOps inserted just before, in the same engine
    stream (engines are in-order, so earlier waits are strictly safe)."""
    n_split = 0
    for f in nc.m.functions:
        for bb in f.blocks:
            insts = list(bb.instructions)
            if not any(
                i.sync_info is not None and len(i.sync_info.on_wait) > limit
                for i in insts
            ):
                continue
            new = []
            for inst in insts:
                si = inst.sync_info
                if si is not None and len(si.on_wait) > limit:
                    waits = list(si.on_wait)
                    for j, w in enumerate(waits[: len(waits) - limit]):
                        nop = mybir.InstNoOp(
                            name=f"{inst.name}-hw{j}", engine=inst.engine,
                            ins=[], outs=[],
                        )
                        nop.sync_info = mybir.SyncInfo(on_wait=[w], on_update=[])
                        new.append(nop)
                        n_split += 1
                    inst.sync_info = mybir.SyncInfo(
                        on_wait=waits[len(waits) - limit:],
                        on_update=list(si.on_update),
                    )
                new.append(inst)
            bb.instructions = new
    return n_split


def build_nc(ring_iters=RING_ITERS):
    _patch_drain()
    nc = bass.Bass()
    x_d = nc.dram_tensor("x", [B, D], f32, kind="ExternalInput")
    mu_d = nc.dram_tensor("mu_b", [128, D], f32, kind="ExternalInput")
    pt_d = nc.dram_tensor("pt", [DP, DC, D], bf16, kind="ExternalInput")
    w_d = nc.dram_tensor("w", [HP, HC, D], f32, kind="ExternalInput")
    wt_d = nc.dram_tensor("wt", [DP, DC, H], f32, kind="ExternalInput")
    rt_d = nc.dram_tensor("rt", [HP, HC, O], bf16, kind="ExternalInput")
    wb_d = nc.dram_tensor("w_bf", [HP, HC, D], bf16, kind="ExternalInput")
    wtb_d = nc.dram_tensor("wt_bf", [DP, DC, H], bf16, kind="ExternalInput")
    bb_d = nc.dram_tensor("b_b", [128, O], f32, kind="ExternalInput")
    out_d = nc.dram_tensor("out", [B, O], f32, kind="ExternalOutput")

    AT = mybir.AluOpType

    with TileContext(nc) as tc, ExitStack() as ctx:
        persist = ctx.enter_context(tc.tile_pool(name="persist", bufs=1))
        xpool = ctx.enter_context(tc.tile_pool(name="xpool", bufs=3))
        small = ctx.enter_context(tc.tile_pool(name="small", bufs=2))
        psA = ctx.enter_context(tc.tile_pool(name="psA", bufs=3, space="PSUM"))
        psB = ctx.enter_context(tc.tile_pool(name="psB", bufs=3, space="PSUM"))
        psT = ctx.enter_context(tc.tile_pool(name="psT", bufs=2, space="PSUM"))

        ident = persist.tile([128, 128], f32, tag="ident")
        make_identity(nc, ident)
        maskSL = persist.tile([K, K], f32, tag="maskSL")
        nc.gpsimd.memset(maskSL, LR)
        # keep lr where row > col (strict lower), else 0
        nc.gpsimd.affine_select(
            out=maskSL, in_=maskSL, compare_op=AT.is_gt, fill=0.0,
            base=0, pattern=[[-1, K]], channel_multiplier=1,
        )

        mu_t = persist.tile([128, D], f32, tag="mu")
        nc.sync.dma_start(out=mu_t, in_=mu_d[:, :])
        pt_t = persist.tile([DP, DC, D], bf16, tag="pt")
        nc.sync.dma_start(out=pt_t, in_=pt_d[:, :, :])
        W = persist.tile([HP, HC, D], f32, tag="W")
        nc.sync.dma_start(out=W, in_=w_d[:, :, :])
        WT = persist.tile([DP, DC, H], f32, tag="WT")
        nc.sync.dma_start(out=WT, in_=wt_d[:, :, :])
        RT = persist.tile([HP, HC, O], bf16, tag="RT")
        nc.sync.dma_start(out=RT, in_=rt_d[:, :, :])
        bb = persist.tile([128, O], f32, tag="bb")
        nc.sync.dma_start(out=bb, in_=bb_d[:, :])
        Wb = persist.tile([HP, HC, D], bf16, tag="Wb")
        nc.sync.dma_start(out=Wb, in_=wb_d[:, :, :])
        WTb = persist.tile([DP, DC, H], bf16, tag="WTb")
        nc.sync.dma_start(out=WTb, in_=wtb_d[:, :, :])

        Xall = persist.tile([DP, NBLK, DC, K], bf16, tag="Xall")
        XTall = persist.tile([K, NBLK, D], f32, tag="XTall")
        Sxxall = persist.tile([K, NBLK, K], f32, tag="Sxxall")

        # ---------------- whitening ----------------
        for bi in range(NBLK):
            xt = xpool.tile([128, D], f32, tag="xraw")
            nc.sync.dma_start(out=xt, in_=x_d[bi * K:(bi + 1) * K, :])
            xc = xpool.tile([128, D], f32, tag="xc")
            nc.vector.tensor_sub(xc, xt, mu_t)
            xctb = xpool.tile([DP, DC, K], bf16, tag="xct")
            for ic in range(DC):
                tp = psT.tile([DP, K], f32, tag="t")
                nc.tensor.transpose(tp, xc[:, ic * DP:(ic + 1) * DP], ident)
                nc.vector.tensor_copy(xctb[:, ic, :], tp)
            XTb = XTall[:, bi, :]
            for s in range(2):
                ps = psB.tile([K, DS], f32, tag="big")
                for ic in range(DC):
                    nc.tensor.matmul(
                        ps, xctb[:, ic, :], pt_t[:, ic, s * DS:(s + 1) * DS],
                        start=(ic == 0), stop=(ic == DC - 1),
                    )
                nc.vector.tensor_add(
                    XTb[:, s * DS:(s + 1) * DS], ps, xc[:, s * DS:(s + 1) * DS]
                )
            for ic in range(DC):
                tp = psT.tile([DP, K], f32, tag="t")
                nc.tensor.transpose(tp, XTb[:, ic * DP:(ic + 1) * DP], ident)
                nc.vector.tensor_copy(Xall[:, bi, ic, :], tp)
            ps = psA.tile([K, K], f32, tag="kk")
            for ic in range(DC):
                nc.tensor.matmul(
                    ps, Xall[:, bi, ic, :], Xall[:, bi, ic, :],
                    start=(ic == 0), stop=(ic == DC - 1),
                )
            nc.vector.tensor_copy(Sxxall[:, bi, :], ps)

        # ---------------- sequential block scan ----------------
        for bi in range(NBLK):
            XTb = XTall[:, bi, :]
            # Y0 [HP, HC, K] = W X
            y0 = small.tile([HP, HC, K], bf16, tag="y0")
            for hc in range(HC):
                ps = psA.tile([HP, K], f32, tag="kk")
                for ic in range(DC):
                    nc.tensor.matmul(
                        ps, WTb[:, ic, hc * HP:(hc + 1) * HP], Xall[:, bi, ic, :],
                        start=(ic == 0), stop=(ic == DC - 1),
                    )
                nc.vector.tensor_copy(y0[:, hc, :], ps)
            # Y0T [K, H] = X^T W^T
            y0t = small.tile([K, H], bf16, tag="y0t")
            ps = psB.tile([K, H], f32, tag="big")
            for ic in range(DC):
                nc.tensor.matmul(
                    ps, Xall[:, bi, ic, :], WTb[:, ic, :],
                    start=(ic == 0), stop=(ic == DC - 1),
                )
            nc.vector.tensor_copy(y0t, ps)
            # Syy [K, K] = Y0^T Y0
            syy = small.tile([K, K], bf16, tag="syy")
            ps = psA.tile([K, K], f32, tag="kk")
            for hc in range(HC):
                nc.tensor.matmul(
                    ps, y0[:, hc, :], y0[:, hc, :],
                    start=(hc == 0), stop=(hc == HC - 1),
                )
            nc.vector.tensor_copy(syy, ps)
            # T0T [K, D] = Y0^T W
            t0t = small.tile([K, D], bf16, tag="t0t")
            for s in range(2):
                ps = psB.tile([K, DS], f32, tag="big")
                for hc in range(HC):
                    nc.tensor.matmul(
                        ps, y0[:, hc, :], Wb[:, hc, s * DS:(s + 1) * DS],
                        start=(hc == 0), stop=(hc == HC - 1),
                    )
                nc.vector.tensor_copy(t0t[:, s * DS:(s + 1) * DS], ps)

            sxx = Sxxall[:, bi, :]
            # ---- fixed-point ring on K x K matrices ----
            A = small.tile([K, K], bf16, tag="A")
            Bm = small.tile([K, K], bf16, tag="B")
            nc.vector.tensor_copy(A, ident)
            nc.vector.tensor_copy(Bm, ident)
            for m in range(ring_iters):
                r1 = psA.tile([K, K], f32, tag="kk")
                nc.tensor.matmul(r1, A, syy, start=True, stop=True)
                s_sb = small.tile([K, K], bf16, tag="S")
                nc.vector.tensor_sub(s_sb, sxx, r1)
                ct = psA.tile([K, K], f32, tag="kk")
                nc.tensor.matmul(ct, s_sb, Bm, start=True, stop=True)
                nt = small.tile([K, K], bf16, tag="NT")
                nc.vector.tensor_mul(nt, ct, maskSL)
                a1 = psA.tile([K, K], f32, tag="kk")
                nc.tensor.matmul(a1, nt, A, start=True, stop=True)
                nc.vector.tensor_add(A, a1, ident)
                z2 = psA.tile([K, K], f32, tag="kk")
                nc.tensor.matmul(z2, syy, A, start=True, stop=True)
                z2s = small.tile([K, K], bf16, tag="Z2")
                nc.vector.tensor_copy(z2s, z2)
                g = psA.tile([K, K], f32, tag="kk")
                nc.tensor.matmul(g, A, z2s, start=True, stop=True)
                gm = small.tile([K, K], bf16, tag="GM")
                nc.vector.tensor_mul(gm, g, maskSL)
                b1 = psA.tile([K, K], f32, tag="kk")
                nc.tensor.matmul(b1, gm, Bm, start=True, stop=True)
                nc.vector.tensor_sub(Bm, ident, b1)

            # ---- epilogue ----
            yt = small.tile([K, H], bf16, tag="yt")
            ps = psB.tile([K, H], f32, tag="big")
            nc.tensor.matmul(ps, A, y0t, start=True, stop=True)
            nc.vector.tensor_copy(yt, ps)
            # logits: relu(Y0 A) then readout
            relu_y = small.tile([HP, HC, K], bf16, tag="reluy")
            for hc in range(HC):
                ps2 = psA.tile([HP, K], f32, tag="kk")
                nc.tensor.matmul(
                    ps2, y0t[:, hc * HP:(hc + 1) * HP], A, start=True, stop=True
                )
                nc.scalar.activation(
                    relu_y[:, hc, :], ps2, mybir.ActivationFunctionType.Relu
                )
            lg = psA.tile([K, O], f32, tag="kk")
            for hc in range(HC):
                nc.tensor.matmul(
                    lg, relu_y[:, hc, :], RT[:, hc, :],
                    start=(hc == 0), stop=(hc == HC - 1),
                )
            lgs = small.tile([K, O], f32, tag="lg")
            nc.vector.tensor_add(lgs, lg, bb)
            nc.sync.dma_start(out=out_d[bi * K:(bi + 1) * K, :], in_=lgs)

            # UT = B^T (XT - A^T T0T)
            q = small.tile([K, D], bf16, tag="q")
            for s in range(2):
                ps3 = psB.tile([K, DS], f32, tag="big")
                nc.tensor.matmul(
                    ps3, A, t0t[:, s * DS:(s + 1) * DS], start=True, stop=True
                )
                nc.vector.tensor_sub(
                    q[:, s * DS:(s + 1) * DS], XTb[:, s * DS:(s + 1) * DS], ps3
                )
            ut = small.tile([K, D], bf16, tag="ut")
            for s in range(2):
                ps4 = psB.tile([K, DS], f32, tag="big")
                nc.tensor.matmul(
                    ps4, Bm, q[:, s * DS:(s + 1) * DS], start=True, stop=True
                )
                nc.vector.tensor_copy(ut[:, s * DS:(s + 1) * DS], ps4)
            # W += lr * Y U^T ; WT += lr * U Y^T
            for hc in range(HC):
                for s in range(2):
                    ps5 = psB.tile([HP, DS], f32, tag="big")
                    nc.tensor.matmul(
                        ps5, yt[:, hc * HP:(hc + 1) * HP],
                        ut[:, s * DS:(s + 1) * DS], start=True, stop=True,
                    )
                    wslice = W[:, hc, s * DS:(s + 1) * DS]
                    wbslice = Wb[:, hc, s * DS:(s + 1) * DS]
                    nc.vector.scalar_tensor_tensor(
                        wbslice, ps5, LR, wslice, op0=AT.mult, op1=AT.add
                    )
                    nc.vector.scalar_tensor_tensor(
                        wslice, ps5, LR, wslice, op0=AT.mult, op1=AT.add
                    )
            for dc in range(DC):
                ps6 = psB.tile([DP, H], f32, tag="big")
                nc.tensor.matmul(
                    ps6, ut[:, dc * DP:(dc + 1) * DP], yt, start=True, stop=True
                )
                wtslice = WT[:, dc, :]
                wtbslice = WTb[:, dc, :]
                nc.vector.scalar_tensor_tensor(
                    wtbslice, ps6, LR, wtslice, op0=AT.mult, op1=AT.add
                )
                nc.vector.scalar_tensor_tensor(
                    wtslice, ps6, LR, wtslice, op0=AT.mult, op1=AT.add
                )

    _split_multiwait(nc)
    return nc


def prep_inputs(x, whiten_mean, whiten_mat, oja_W, readout_W, readout_b):
    """Host-side layout/dtype prep (no contractions)."""
    x = np.ascontiguousarray(x, dtype=np.float32)
    mu_b = np.broadcast_to(
        np.asarray(whiten_mean, dtype=np.float32)[None, :], (128, D)
    ).copy()
    P = np.asarray(whiten_mat, dtype=np.float32) - np.eye(D, dtype=np.float32)
    # pt[dp, ic, dout] = P^T[ic*112+dp, dout] = P[dout, ic*112+dp]
    pt = np.ascontiguousarray(
        P.T.reshape(DC, DP, D).transpose(1, 0, 2).astype(ml_dtypes.bfloat16)
    )
    Wf = np.asarray(oja_W, dtype=np.float32)
    w = np.ascontiguousarray(Wf.reshape(HC, HP, D).transpose(1, 0, 2))
    wt = np.ascontiguousarray(Wf.T.reshape(DC, DP, H).transpose(1, 0, 2))
    Rf = np.asarray(readout_W, dtype=np.float32)
    rt = np.ascontiguousarray(
        Rf.T.reshape(HC, HP, O).transpose(1, 0, 2).astype(ml_dtypes.bfloat16)
    )
    b_b = np.broadcast_to(
        np.asarray(readout_b, dtype=np.float32)[None, :], (128, O)
    ).copy()
    return {
        "x": x, "mu_b": mu_b, "pt": pt, "w": w, "wt": wt, "rt": rt, "b_b": b_b,
        "w_bf": w.astype(ml_dtypes.bfloat16), "wt_bf": wt.astype(ml_dtypes.bfloat16),
    }


_cached_nc = None


def _get_nc():
    global _cached_nc
    if _cached_nc is None:
        _cached_nc = build_nc()
    return _cached_nc


def kernel(x, whiten_mean, whiten_mat, oja_W, readout_W, readout_b, **run_kwargs):
    nc = _get_nc()
    ins = prep_inputs(x, whiten_mean, whiten_mat, oja_W, readout_W, readout_b)
    res = run_bass_kernel_spmd(
        nc, [ins] * N_CORES, core_ids=list(range(N_CORES)), **run_kwargs
    )
    out = res.results[0]["out"]
    if run_kwargs:
        kernel.last_result = res
    return out


# revision 7
# speedup vs baseline: 1.7954x; 1.0497x over previous
"""Trainium2 Bass kernel for nn_BioClassifier: whitening + sequential Oja scan + readout.

Algorithm: chunk the 2048-sample sequential Oja scan into 16 blocks of K=128.
Within a block starting at weight W0 with whitened samples X [D,K] (XT [K,D]):
    Y = Y0 A,  U = (X - T0 A) B
    A = (I - lr*SU(C))^-1,  B = (I + lr*SU(G))^-1
    C = U^T X,  G = Y^T Y,   (SU = strict upper triangle)
and the key identity T0^T X = Y0^T W0 X = Y0^T Y0 = Syy, so the (A,B,C,G)
fixed point closes over K x K matrices only:
    S  = Sxx - A^T Syy
    CT = S^T B              (computed transposed: lhsT = S)
    A  = I + (lr*SL(CT))^T A
    G  = A^T (Syy A)
    B  = I - (lr*SL(G))^T B (G symmetric)
Iterated RING_ITERS times (geometric convergence ~0.37/iter), then:
    YT = A^T Y0T (feats), UT = B^T (XT - A^T T0T)
    W += lr * YT^T UT, WT += lr * UT^T YT  (both layouts kept in SBUF)
    logits = relu(Y0 A)^T-path @ readout + bias
Whitening uses xw = xc + xc @ (M - I)^T with (M - I) in bf16 (M ~ I + 0.01*N,
so the bf16 term is a small correction; xc added back in fp32).
All heavy compute in fp32 matmuls unless MM_DT overrides.
"""

import os
import sys
from contextlib import ExitStack

sys.path.insert(0, "/opt/trn_rl_repo")

import numpy as np
import ml_dtypes

import concourse.bass as bass
import concourse.mybir as mybir
from concourse.tile import TileContext
from concourse.masks import make_identity
from concourse.bass_utils import run_bass_kernel_spmd
from concourse.vector_clock import ScopedClock

LR = 1e-3
B, D, H, O = 2048, 784, 256, 10
K = 128
NBLK = B // K
DP, DC = 112, 7          # D = 784 = 7 * 112
HP, HC = 128, 2          # H = 256 = 2 * 128
DS = D // 2              # 392: matmul free-dim split for D-wide outputs

RING_ITERS = int(os.environ.get("RING_ITERS", "5"))
N_CORES = 8

f32 = mybir.dt.float32
bf16 = mybir.dt.bfloat16

def _install_ntff_hook():
    """The agent image's `antenv` lacks `axon_hooks`, so trace=True degrades.
    Synthesize the module and register the ctypes NTFF hook from trn_boot."""
    import types
    import antenv

    if getattr(antenv, "axon_hooks", None) is not None:
        return
    mod = types.ModuleType("antenv.axon_hooks")
    _hook_box = [None]
    mod.set_axon_ntff_profile_hook = lambda h: _hook_box.__setitem__(0, h)
    mod.get_axon_ntff_profile_hook = lambda: _hook_box[0]
    sys.modules["antenv.axon_hooks"] = mod
    antenv.axon_hooks = mod
    try:
        sys.path.insert(0, "/root/.axon_site")
        from trn_agent_boot.trn_boot import _ntff_profile_via_ctypes

        hook = _ntff_profile_via_ctypes("/opt/axon/libaxon_pjrt.so")
        if hook is not None:
            mod.set_axon_ntff_profile_hook(hook)
    except Exception:
        pass


try:
    _install_ntff_hook()
except Exception:
    pass

_drain_patched = False


def _patch_drain():
    """This walrus build only supports one sync-wait per CTRL instruction;
    split the Tile kernel-tail drain into one drain per semaphore wait."""
    global _drain_patched
    if _drain_patched:
        return

    def patched(self, tick_clock, wait_clock):
        drain_inst = self.nc.sync.drain()
        wait_clock.add_sem_waits(
            drain_inst.ins, ScopedClock({None: tick_clock.global_clock})
        )
        mi = drain_inst.ins
        si = mi.sync_info
        if si is not None and len(si.on_wait) > 1:
            waits = list(si.on_wait)
            mi.sync_info = mybir.SyncInfo(
                on_wait=[waits[0]], on_update=list(si.on_update)
            )
            for w in waits[1:]:
                d2 = self.nc.sync.drain()
                d2.ins.sync_info = mybir.SyncInfo(on_wait=[w], on_update=[])
        self.nc.all_engine_barrier()
        assert self.sems is not None
        popped = self.nc._tile_sem_poison_stack.pop()
        assert popped is self._sem_poison
        self.nc.clear_and_free_semaphores(list(self.sems.allocated().values()))
        self.nc.all_engine_barrier()

    TileContext._drain_and_barrier = patched
    _drain_patched = True


def _split_multiwait(nc, limit=1):
    """This walrus build supports only `limit` sync-waits per instruction.
    Hoist extra waits onto NoOps inserted just before, in the same engine
    stream (engines are in-order, so earlier waits are strictly safe)."""
    n_split = 0
    for f in nc.m.functions:
        for bb in f.blocks:
            insts = list(bb.instructions)
            if not any(
                i.sync_info is not None and len(i.sync_info.on_wait) > limit
                for i in insts
            ):
                continue
            new = []
            for inst in insts:
                si = inst.sync_info
                if si is not None and len(si.on_wait) > limit:
                    waits = list(si.on_wait)
                    for j, w in enumerate(waits[: len(waits) - limit]):
                        nop = mybir.InstNoOp(
                            name=f"{inst.name}-hw{j}", engine=inst.engine,
                            ins=[], outs=[],
                        )
                        nop.sync_info = mybir.SyncInfo(on_wait=[w], on_update=[])
                        new.append(nop)
                        n_split += 1
                    inst.sync_info = mybir.SyncInfo(
                        on_wait=waits[len(waits) - limit:],
                        on_update=list(si.on_update),
                    )
                new.append(inst)
            bb.instructions = new
    return n_split


def build_nc(ring_iters=RING_ITERS):
    _patch_drain()
    nc = bass.Bass()
    x_d = nc.dram_tensor("x", [B, D], f32, kind="ExternalInput")
    mu_d = nc.dram_tensor("mu_b", [128, D], f32, kind="ExternalInput")
    pt_d = nc.dram_tensor("pt", [DP, DC, D], bf16, kind="ExternalInput")
    w_d = nc.dram_tensor("w", [HP, HC, D], f32, kind="ExternalInput")
    wt_d = nc.dram_tensor("wt", [DP, DC, H], f32, kind="ExternalInput")
    rt_d = nc.dram_tensor("rt", [HP, HC, O], bf16, kind="ExternalInput")
    wb_d = nc.dram_tensor("w_bf", [HP, HC, D], bf16, kind="ExternalInput")
    wtb_d = nc.dram_tensor("wt_bf", [DP, DC, H], bf16, kind="ExternalInput")
    bb_d = nc.dram_tensor("b_b", [128, O], f32, kind="ExternalInput")
    out_d = nc.dram_tensor("out", [B, O], f32, kind="ExternalOutput")

    AT = mybir.AluOpType

    with TileContext(nc) as tc, ExitStack() as ctx:
        persist = ctx.enter_context(tc.tile_pool(name="persist", bufs=1))
        xpool = ctx.enter_context(tc.tile_pool(name="xpool", bufs=3))
        small = ctx.enter_context(tc.tile_pool(name="small", bufs=2))
        psA = ctx.enter_context(tc.tile_pool(name="psA", bufs=3, space="PSUM"))
        psB = ctx.enter_context(tc.tile_pool(name="psB", bufs=3, space="PSUM"))
        psT = ctx.enter_context(tc.tile_pool(name="psT", bufs=2, space="PSUM"))

        ident = persist.tile([128, 128], f32, tag="ident")
        make_identity(nc, ident)
        maskSL = persist.tile([K, K], f32, tag="maskSL")
        nc.gpsimd.memset(maskSL, LR)
        # keep lr where row > col (strict lower), else 0
        nc.gpsimd.affine_select(
            out=maskSL, in_=maskSL, compare_op=AT.is_gt, fill=0.0,
            base=0, pattern=[[-1, K]], channel_multiplier=1,
        )

        mu_t = persist.tile([128, D], f32, tag="mu")
        nc.sync.dma_start(out=mu_t, in_=mu_d[:, :])
        pt_t = persist.tile([DP, DC, D], bf16, tag="pt")
        nc.sync.dma_start(out=pt_t, in_=pt_d[:, :, :])
        W = persist.tile([HP, HC, D], f32, tag="W")
        nc.sync.dma_start(out=W, in_=w_d[:, :, :])
        WT = persist.tile([DP, DC, H], f32, tag="WT")
        nc.sync.dma_start(out=WT, in_=wt_d[:, :, :])
        RT = persist.tile([HP, HC, O], bf16, tag="RT")
        nc.sync.dma_start(out=RT, in_=rt_d[:, :, :])
        bb = persist.tile([128, O], f32, tag="bb")
        nc.sync.dma_start(out=bb, in_=bb_d[:, :])
        Wb = persist.tile([HP, HC, D], bf16, tag="Wb")
        nc.sync.dma_start(out=Wb, in_=wb_d[:, :, :])
        WTb = persist.tile([DP, DC, H], bf16, tag="WTb")
        nc.sync.dma_start(out=WTb, in_=wtb_d[:, :, :])

        Xall = persist.tile([DP, NBLK, DC, K], bf16, tag="Xall")
        XTall = persist.tile([K, NBLK, D], f32, tag="XTall")
        Sxxall = persist.tile([K, NBLK, K], f32, tag="Sxxall")

        # ---------------- whitening (emitted interleaved with scan) ----------
        def whiten(bi):
            xt = xpool.tile([128, D], f32, tag="xraw")
            nc.sync.dma_start(out=xt, in_=x_d[bi * K:(bi + 1) * K, :])
            xc = xpool.tile([128, D], f32, tag="xc")
            nc.vector.tensor_sub(xc, xt, mu_t)
            xctb = xpool.tile([DP, DC, K], bf16, tag="xct")
            for ic in range(DC):
                tp = psT.tile([DP, K], f32, tag="t")
                nc.tensor.transpose(tp, xc[:, ic * DP:(ic + 1) * DP], ident)
                nc.scalar.copy(xctb[:, ic, :], tp)
            XTb = XTall[:, bi, :]
            for s in range(2):
                ps = psB.tile([K, DS], f32, tag="big")
                for ic in range(DC):
                    nc.tensor.matmul(
                        ps, xctb[:, ic, :], pt_t[:, ic, s * DS:(s + 1) * DS],
                        start=(ic == 0), stop=(ic == DC - 1),
                    )
                nc.vector.tensor_add(
                    XTb[:, s * DS:(s + 1) * DS], ps, xc[:, s * DS:(s + 1) * DS]
                )
            for ic in range(DC):
                tp = psT.tile([DP, K], f32, tag="t")
                nc.tensor.transpose(tp, XTb[:, ic * DP:(ic + 1) * DP], ident)
                nc.scalar.copy(Xall[:, bi, ic, :], tp)
            ps = psA.tile([K, K], f32, tag="kk")
            for ic in range(DC):
                nc.tensor.matmul(
                    ps, Xall[:, bi, ic, :], Xall[:, bi, ic, :],
                    start=(ic == 0), stop=(ic == DC - 1),
                )
            nc.vector.tensor_copy(Sxxall[:, bi, :], ps)

        LOOKAHEAD = 3
        for bi in range(LOOKAHEAD):
            whiten(bi)

        # ---------------- sequential block scan ----------------
        for bi in range(NBLK):
            if bi + LOOKAHEAD < NBLK:
                whiten(bi + LOOKAHEAD)
            XTb = XTall[:, bi, :]
            # Y0 [HP, HC, K] = W X
            y0 = small.tile([HP, HC, K], bf16, tag="y0")
            for hc in range(HC):
                ps = psA.tile([HP, K], f32, tag="kk")
                for ic in range(DC):
                    nc.tensor.matmul(
                        ps, WTb[:, ic, hc * HP:(hc + 1) * HP], Xall[:, bi, ic, :],
                        start=(ic == 0), stop=(ic == DC - 1),
                    )
                nc.vector.tensor_copy(y0[:, hc, :], ps)
            # Y0T [K, H] = X^T W^T
            y0t = small.tile([K, H], bf16, tag="y0t")
            ps = psB.tile([K, H], f32, tag="big")
            for ic in range(DC):
                nc.tensor.matmul(
                    ps, Xall[:, bi, ic, :], WTb[:, ic, :],
                    start=(ic == 0), stop=(ic == DC - 1),
                )
            nc.vector.tensor_copy(y0t, ps)
            # Syy [K, K] = Y0^T Y0
            syy = small.tile([K, K], bf16, tag="syy")
            ps = psA.tile([K, K], f32, tag="kk")
            for hc in range(HC):
                nc.tensor.matmul(
                    ps, y0[:, hc, :], y0[:, hc, :],
                    start=(hc == 0), stop=(hc == HC - 1),
                )
            nc.vector.tensor_copy(syy, ps)
            # T0T [K, D] = Y0^T W
            t0t = small.tile([K, D], bf16, tag="t0t")
            for s in range(2):
                ps = psB.tile([K, DS], f32, tag="big")
                for hc in range(HC):
                    nc.tensor.matmul(
                        ps, y0[:, hc, :], Wb[:, hc, s * DS:(s + 1) * DS],
                        start=(hc == 0), stop=(hc == HC - 1),
                    )
                nc.vector.tensor_copy(t0t[:, s * DS:(s + 1) * DS], ps)

            sxx = Sxxall[:, bi, :]
            # ---- fixed-point ring on K x K matrices ----
            A = small.tile([K, K], bf16, tag="A")
            Bm = small.tile([K, K], bf16, tag="B")
            nc.vector.tensor_copy(A, ident)
            nc.vector.tensor_copy(Bm, ident)
            for m in range(ring_iters):
                r1 = psA.tile([K, K], f32, tag="kk")
                nc.tensor.matmul(r1, A, syy, start=True, stop=True)
                s_sb = small.tile([K, K], bf16, tag="S")
                nc.vector.tensor_sub(s_sb, sxx, r1)
                ct = psA.tile([K, K], f32, tag="kk")
                nc.tensor.matmul(ct, s_sb, Bm, start=True, stop=True)
                nt = small.tile([K, K], bf16, tag="NT")
                nc.vector.tensor_mul(nt, ct, maskSL)
                a1 = psA.tile([K, K], f32, tag="kk")
                nc.tensor.matmul(a1, nt, A, start=True, stop=True)
                nc.vector.tensor_add(A, a1, ident)
                z2 = psA.tile([K, K], f32, tag="kk")
                nc.tensor.matmul(z2, syy, A, start=True, stop=True)
                z2s = small.tile([K, K], bf16, tag="Z2")
                nc.vector.tensor_copy(z2s, z2)
                g = psA.tile([K, K], f32, tag="kk")
                nc.tensor.matmul(g, A, z2s, start=True, stop=True)
                gm = small.tile([K, K], bf16, tag="GM")
                nc.vector.tensor_mul(gm, g, maskSL)
                b1 = psA.tile([K, K], f32, tag="kk")
                nc.tensor.matmul(b1, gm, Bm, start=True, stop=True)
                nc.vector.tensor_sub(Bm, ident, b1)

            # ---- epilogue ----
            yt = small.tile([K, H], bf16, tag="yt")
            ps = psB.tile([K, H], f32, tag="big")
            nc.tensor.matmul(ps, A, y0t, start=True, stop=True)
            nc.vector.tensor_copy(yt, ps)
            # logits: relu(Y0 A) then readout
            relu_y = small.tile([HP, HC, K], bf16, tag="reluy")
            for hc in range(HC):
                ps2 = psA.tile([HP, K], f32, tag="kk")
                nc.tensor.matmul(
                    ps2, y0t[:, hc * HP:(hc + 1) * HP], A, start=True, stop=True
                )
                nc.scalar.activation(
                    relu_y[:, hc, :], ps2, mybir.ActivationFunctionType.Relu
                )
            lg = psA.tile([K, O], f32, tag="kk")
            for hc in range(HC):
                nc.tensor.matmul(
                    lg, relu_y[:, hc, :], RT[:, hc, :],
                    start=(hc == 0), stop=(hc == HC - 1),
                )
            lgs = small.tile([K, O], f32, tag="lg")
            nc.vector.tensor_add(lgs, lg, bb)
            nc.sync.dma_start(out=out_d[bi * K:(bi + 1) * K, :], in_=lgs)

            # UT = B^T (XT - A^T T0T)
            q = small.tile([K, D], bf16, tag="q")
            for s in range(2):
                ps3 = psB.tile([K, DS], f32, tag="big")
                nc.tensor.matmul(
                    ps3, A, t0t[:, s * DS:(s + 1) * DS], start=True, stop=True
                )
                nc.vector.tensor_sub(
                    q[:, s * DS:(s + 1) * DS], XTb[:, s * DS:(s + 1) * DS], ps3
                )
            ut = small.tile([K, D], bf16, tag="ut")
            for s in range(2):
                ps4 = psB.tile([K, DS], f32, tag="big")
                nc.tensor.matmul(
                    ps4, Bm, q[:, s * DS:(s + 1) * DS], start=True, stop=True
                )
                nc.vector.tensor_copy(ut[:, s * DS:(s + 1) * DS], ps4)
            # W += lr * Y U^T ; WT += lr * U Y^T
            for hc in range(HC):
                for s in range(2):
                    ps5 = psB.tile([HP, DS], f32, tag="big")
                    nc.tensor.matmul(
                        ps5, yt[:, hc * HP:(hc + 1) * HP],
                        ut[:, s * DS:(s + 1) * DS], start=True, stop=True,
                    )
                    wslice = W[:, hc, s * DS:(s + 1) * DS]
                    wbslice = Wb[:, hc, s * DS:(s + 1) * DS]
                    nc.vector.scalar_tensor_tensor(
                        wbslice, ps5, LR, wslice, op0=AT.mult, op1=AT.add
                    )
                    nc.vector.scalar_tensor_tensor(
                        wslice, ps5, LR, wslice, op0=AT.mult, op1=AT.add
                    )
            for dc in range(DC):
                ps6 = psB.tile([DP, H], f32, tag="big")
                nc.tensor.matmul(
                    ps6, ut[:, dc * DP:(dc + 1) * DP], yt, start=True, stop=True
                )
                wtslice = WT[:, dc, :]
                wtbslice = WTb[:, dc, :]
                nc.vector.scalar_tensor_tensor(
                    wtbslice, ps6, LR, wtslice, op0=AT.mult, op1=AT.add
                )
                nc.vector.scalar_tensor_tensor(
                    wtslice, ps6, LR, wtslice, op0=AT.mult, op1=AT.add
                )

    _split_multiwait(nc)
    return nc


def prep_inputs(x, whiten_mean, whiten_mat, oja_W, readout_W, readout_b):
    """Host-side layout/dtype prep (no contractions)."""
    x = np.ascontiguousarray(x, dtype=np.float32)
    mu_b = np.broadcast_to(
        np.asarray(whiten_mean, dtype=np.float32)[None, :], (128, D)
    ).copy()
    P = np.asarray(whiten_mat, dtype=np.float32) - np.eye(D, dtype=np.float32)
    # pt[dp, ic, dout] = P^T[ic*112+dp, dout] = P[dout, ic*112+dp]
    pt = np.ascontiguousarray(
        P.T.reshape(DC, DP, D).transpose(1, 0, 2).astype(ml_dtypes.bfloat16)
    )
    Wf = np.asarray(oja_W, dtype=np.float32)
    w = np.ascontiguousarray(Wf.reshape(HC, HP, D).transpose(1, 0, 2))
    wt = np.ascontiguousarray(Wf.T.reshape(DC, DP, H).transpose(1, 0, 2))
    Rf = np.asarray(readout_W, dtype=np.float32)
    rt = np.ascontiguousarray(
        Rf.T.reshape(HC, HP, O).transpose(1, 0, 2).astype(ml_dtypes.bfloat16)
    )
    b_b = np.broadcast_to(
        np.asarray(readout_b, dtype=np.float32)[None, :], (128, O)
    ).copy()
    return {
        "x": x, "mu_b": mu_b, "pt": pt, "w": w, "wt": wt, "rt": rt, "b_b": b_b,
        "w_bf": w.astype(ml_dtypes.bfloat16), "wt_bf": wt.astype(ml_dtypes.bfloat16),
    }


_cached_nc = None


def _get_nc():
    global _cached_nc
    if _cached_nc is None:
        _cached_nc = build_nc()
    return _cached_nc


def kernel(x, whiten_mean, whiten_mat, oja_W, readout_W, readout_b, **run_kwargs):
    nc = _get_nc()
    ins = prep_inputs(x, whiten_mean, whiten_mat, oja_W, readout_W, readout_b)
    res = run_bass_kernel_spmd(
        nc, [ins] * N_CORES, core_ids=list(range(N_CORES)), **run_kwargs
    )
    out = res.results[0]["out"]
    if run_kwargs:
        kernel.last_result = res
    return out
